# revision 48
# baseline (speedup 1.0000x reference)
"""Trainium2 Bass kernel for nn_MultiHeadAttention (B=2, E=1024, S=2048, H=16).

Sharding: 8 cores = 2 batches x 4 head-groups (4 heads = 256 channels each).
Each core computes its head-group's QKV projections, attention, and a partial
output projection over its 256 channels; the host sums the 4 partials per
batch and adds the host-folded constant (Wo @ bv + bo).

Numerics:
- bf16 matmul inputs everywhere with fp32 PSUM accumulation. (fp8 DoubleRow
  was tried and is ~1.5-2x faster on PE, but attention output is a weighted
  mean over ~300 effective keys, so quantization noise on scores/weights/
  values propagates ~1:1 into the output: measured 6e-2 rel err vs the 2e-2
  gate. bf16 keeps it at ~4e-3.)
- Softmax without max-subtraction; the additive mask becomes a multiplicative
  exp(mask) (host-precomputed, bf16). exp on ACT -> bf16, mask multiply on
  DVE -> bf16 weights. The softmax denominator comes from an appended
  ones-column in the attn@V matmul; division is den-copy + fast-reciprocal
  on DVE + partition_broadcast on Pool + multiply on DVE (no PE/PSUM).

Engine balance: ACT does only the exp stream (the bottleneck); projection
bias/scale and output-projection PSUM drains go to Pool (gpsimd); the mask
multiplies are split DVE/Pool. Emission interleaves the projections and the
output projection into the attention stream to keep PE dense (p-state ramp).

All DRAM tensors are host-packed so every DMA is contiguous.
"""

import os
from contextlib import ExitStack

import numpy as np
import ml_dtypes

import concourse.bass as bass
import concourse.tile as tile
from concourse import bacc, mybir
from concourse import bass_utils

BF16 = mybir.dt.bfloat16
F32 = mybir.dt.float32
Exp = mybir.ActivationFunctionType.Exp
ADD = mybir.AluOpType.add
MULT = mybir.AluOpType.mult

B, E, S, H = 2, 1024, 2048, 16
DH = E // H                      # 64
NCORES = 8
GROUPS = 4                       # head groups (cores per batch)
HPC = H // GROUPS                # 4 heads per core
CH = HPC * DH                    # 256 channels per core
A = CH // 128                    # 2 partition chunks of channels
KE = E // 128                    # 8 contraction chunks over E
QB = S // 512                    # 4 q-blocks of 512
QP2 = S // 1024                  # 2 q-block-pairs of 1024
KC = S // 128                    # 16 k-chunks of 128
KT = KC // 2                     # 8 k-chunk pairs (256 rows)
OC = E // 128                    # 8 output-channel chunks

SCALE = float(DH) ** -0.5

# every POOL_MUL_EVERY-th mask-multiply runs on Pool instead of DVE
POOL_MUL_NUM = int(os.environ.get("MHA_POOL_MUL_NUM", "2"))
POOL_MUL_DEN = int(os.environ.get("MHA_POOL_MUL_DEN", "16"))
DELAY_N = int(os.environ.get("MHA_DELAY", "12"))
PSA_BUFS = int(os.environ.get("MHA_PSA", "3"))
PSB_BUFS = int(os.environ.get("MHA_PSB", "1"))
NORM_PE = int(os.environ.get("MHA_NORM_PE", "0"))


def _emit(tc, nc, d, phases="ABC", blevel=4):
    ctx = tc._emit_ctx

    const = ctx.enter_context(tc.tile_pool(name="const", bufs=1))
    xs_pool = ctx.enter_context(tc.tile_pool(name="xs", bufs=16))
    em_pool = ctx.enter_context(tc.tile_pool(name="em", bufs=18))
    w_pool = ctx.enter_context(tc.tile_pool(name="wx", bufs=4))
    sm_pool = ctx.enter_context(tc.tile_pool(name="sm", bufs=3))
    out_pool = ctx.enter_context(tc.tile_pool(name="outp", bufs=3))
    xv_pool = ctx.enter_context(tc.tile_pool(name="xv", bufs=4))
    ps_pool = ctx.enter_context(tc.tile_pool(name="ps", bufs=2, space="PSUM"))
    psb_pool = ctx.enter_context(tc.tile_pool(name="psb", bufs=PSB_BUFS, space="PSUM"))
    psa_pool = ctx.enter_context(tc.tile_pool(name="psa", bufs=PSA_BUFS, space="PSUM"))

    if "Z" in phases:
        zt = const.tile([128, 64], F32, name="zt")
        nc.vector.memset(zt[:], 0.0)
        return
    # ---- resident constants ----
    # DMA issue order matters for the phase-A critical path: k/q weights and
    # biases first (the k/q projections gate the first scores), wo and the
    # remaining masks last.
    wk_sb = const.tile([128, KE, CH], BF16)
    nc.sync.dma_start(wk_sb[:], d["wkT"])
    bk_sb = const.tile([128, A], F32)
    nc.sync.dma_start(bk_sb[:], d["bk"])
    wq_sb = const.tile([128, KE, CH], BF16)
    nc.sync.dma_start(wq_sb[:], d["wqT"])
    bq_sb = const.tile([128, A], F32)
    nc.sync.dma_start(bq_sb[:], d["bq"])
    wv_sb = const.tile([128, KE, CH], BF16)
    wo_sb = const.tile([128, A, E], BF16)

    qp_sb = const.tile([128, A, S], BF16)
    kp_sb = const.tile([128, A, S], BF16)
    vT_sb = const.tile([128, KC, HPC * (DH + 1)], BF16)
    attn_sb = const.tile([128, A, S], BF16)

    for h in range(HPC):
        nc.vector.memset(vT_sb[:, :, 65 * h + 64 : 65 * h + 65], 1.0)
    ones_sb = const.tile([128, 64], BF16)
    nc.vector.memset(ones_sb[:], 1.0)


    # timing-variant support: init tensors a skipped phase would have produced
    if "A" not in phases and "B" in phases:
        nc.vector.memset(qp_sb[:], 0.0)
        nc.vector.memset(kp_sb[:], 0.0)
        nc.vector.memset(vT_sb[:], 1.0)
    if ("B" not in phases or blevel < 4) and "C" in phases:
        nc.vector.memset(attn_sb[:], 0.0)

    # ---- emit helpers ----
    def proj_dma(name, qp2, eng=None):
        xts = []
        for m in range(KE):
            xt = xs_pool.tile([128, 1024], BF16, tag="xq", name="xt")
            (eng or nc.sync).dma_start(xt[:], d[name][qp2, m])
            xts.append(xt)
        return xts

    def proj_unit(w_sb, b_sb, scale2, o_sb, qp2, xts, half, a):
        # one (512-q window, 128-channel chunk) projection: 4 DR matmuls into
        # a single 1-bank PSUM tile + one bias/scale drain on Pool
        ps = psb_pool.tile([128, 512], F32, tag="po", name="ps_pr")
        for m in range(KE):
            nc.tensor.matmul(
                ps[:],
                w_sb[:, m, 128 * a : 128 * (a + 1)],
                xts[m][:, 512 * half : 512 * (half + 1)],
                start=(m == 0),
                stop=(m == KE - 1),
            )
        nc.vector.tensor_scalar(
            o_sb[:, a, 1024 * qp2 + 512 * half : 1024 * qp2 + 512 * (half + 1)],
            ps[:],
            b_sb[:, a : a + 1],
            scale2,
            op0=ADD,
            op1=MULT,
        )

    def proj_units(w_sb, b_sb, scale2, o_sb, qp2, xts):
        return [
            (lambda h=h, a=a: proj_unit(w_sb, b_sb, scale2, o_sb, qp2, xts, h, a))
            for h in range(2)
            for a in range(A)
        ]

    def proj(name, w_sb, b_sb, scale2, o_sb, qp2, xts=None):
        if xts is None:
            xts = proj_dma(name, qp2)
        for u in proj_units(w_sb, b_sb, scale2, o_sb, qp2, xts):
            u()

    xv_tiles = {}

    def xv_dma(kt, eng=None):
        xvt = xv_pool.tile([128, KE, 256], BF16, tag="xv", name=f"xv{kt}")
        (eng or nc.sync).dma_start(xvt[:], d["xv"][kt])
        xv_tiles[kt] = xvt

    def vproj_half(kt, half):
        # vT: v in transposed layout: vT[kpos, c] = sum_e v[e,kpos] WvT[e,c]
        ps_v = psb_pool.tile([128, 512], F32, tag="po", name="ps_v")
        for m in range(KE):
            nc.tensor.matmul(
                ps_v[:, 0:CH],
                xv_tiles[kt][:, m, 128 * half : 128 * (half + 1)],
                wv_sb[:, m, :],
                start=(m == 0),
                stop=(m == KE - 1),
            )
        kc = 2 * kt + half
        nc.vector.tensor_copy(
            vT_sb[:, kc, :].rearrange("p (h c) -> p h c", h=HPC)[:, :, 0:DH],
            ps_v[:, 0:CH].rearrange("p (h c) -> p h c", h=HPC),
        )
        # prefetch the chunk 4 slots ahead once this chunk is fully consumed
        # (Pool SWDGE queue so it isn't stuck behind the SP mask backlog)
        if half == 1 and kt + 4 < KT:
            xv_dma(kt + 4, eng=nc.gpsimd)

    def vproj_units():
        return [
            (lambda kt=kt, h=h: vproj_half(kt, h))
            for kt in range(KT)
            for h in range(2)
        ]

    def outproj(oc, qb, copy_eng=None):
        # 512-q granularity so units interleave into the attention stream
        ps_o = psb_pool.tile([128, 512], F32, tag="po", name="ps_o")
        for a in range(A):
            nc.tensor.matmul(
                ps_o[:],
                wo_sb[:, a, 128 * oc : 128 * (oc + 1)],
                attn_sb[:, a, 512 * qb : 512 * (qb + 1)],
                start=(a == 0),
                stop=(a == A - 1),
            )
        ot = out_pool.tile([128, 512], F32, name="ot")
        nc.vector.tensor_copy(ot[:], ps_o[:])
        nc.gpsimd.dma_start(d["out"][oc, :, 512 * qb : 512 * (qb + 1)], ot[:])

    # Deferred-emission queue: keeps PE fed with ready work while the
    # scores->exp->mul chain of recent tiles is still in flight.
    DELAY = DELAY_N
    deferred = []

    def push(fn):
        deferred.append(fn)
        if len(deferred) > DELAY:
            deferred.pop(0)()

    def flush(bg=()):
        bg = list(bg)
        while deferred or bg:
            if deferred:
                deferred.pop(0)()
            if bg:
                bg.pop(0)()

    def attnv_block(attn_t, a, j, tc2, wts):
        # 8 consecutive matmuls of one accumulation group (t-chunk tc2).
        # Entering an open accumulation group costs ~0.6us on HW (PE pipeline
        # + LDW round-trip), so attn@V runs as 4 blocks per (a,j) group
        # instead of interleaved tile-by-tile with the scores stream.
        h = 2 * a + j
        for t in range(4 * tc2, 4 * tc2 + 4):
            for half in range(2):
                nc.tensor.matmul(
                    attn_t[:],
                    vT_sb[:, 2 * t + half, 65 * h : 65 * h + 65],
                    wts[(t, j)][:, 512 * half : 512 * (half + 1)],
                    start=(t == 0 and half == 0),
                    stop=(t == KT - 1 and half == 1),
                )

    def norm(attn_t, a, j, qb):
        den = sm_pool.tile([1, 512], F32, tag="den", name="den")
        nc.vector.tensor_copy(den[:], attn_t[64:65, :])
        rec = sm_pool.tile([1, 512], F32, tag="rec", name="rec")
        nc.vector.reciprocal_approx_fast(rec[:], den[:])
        rb_sb = sm_pool.tile([64, 512], F32, tag="rb", name="rb_sb")
        nc.gpsimd.partition_broadcast(rb_sb[:], rec[:])
        nc.vector.tensor_mul(
            attn_sb[64 * j : 64 * (j + 1), a, 512 * qb : 512 * (qb + 1)],
            attn_t[0:DH, :],
            rb_sb[:],
        )

    mul_idx = [0]
    em_tiles = {}

    def em_prefetch(qb):
        # mask tiles stream on the Pool SWDGE queue so they don't contend
        # with the x/w loads on the SP/ACT queues (em[0,0] stays on SP: it
        # gates the very first mask multiply)
        for t in range(KT):
            e = em_pool.tile([128, 1024], BF16, tag="em", name=f"em{t}_{qb}")
            eng = nc.sync if (t == 0 and qb == 0) else nc.gpsimd
            eng.dma_start(e[:], d["emask"][t, qb])
            em_tiles[(t, qb)] = e

    # slots within a 16-tile group reserved for attnv blocks / norms
    _RESERVED = {2, 4, 5, 6, 10, 12}

    def attention(bg, bg_append):
        # groups of 16 score-tiles; each group's attn@V runs as two blocks in
        # its own tail (t0-3) and two early in the next group (t4-7), far
        # enough behind the exp/mul stream that PE never waits on a mul.
        bg = list(bg)
        groups = [(qb, a) for qb in range(QB) for a in range(A)]
        prev = prev_wts = None
        for gi, (qb, a) in enumerate(groups):
            for fn in bg_append.get(gi, ()):
                bg.extend(fn())
            em2 = [em_tiles[(t, qb)] for t in range(KT)]
            attn_ps = [
                psa_pool.tile([DH + 1, 512], F32, tag="attn", name=f"attn_ps{j}")
                for j in range(2)
            ] if blevel >= 3 else [None, None]
            wts = {}
            for t in range(KT):
                for j in range(2):
                    slot = 2 * t + j
                    if blevel >= 3:
                        if prev is not None:
                            p_ps, p_a, p_qb = prev
                            if slot == 2:
                                attnv_block(p_ps[0], p_a, 0, 1, prev_wts)
                            elif slot == 4:
                                attnv_block(p_ps[1], p_a, 1, 1, prev_wts)
                            elif slot == 5 and blevel >= 4:
                                norm(p_ps[0], p_a, 0, p_qb)
                            elif slot == 6 and blevel >= 4:
                                norm(p_ps[1], p_a, 1, p_qb)
                        if slot == 10:
                            attnv_block(attn_ps[0], a, 0, 0, wts)
                        elif slot == 12:
                            attnv_block(attn_ps[1], a, 1, 0, wts)
                    if bg and (slot not in _RESERVED or (prev is None and slot < 8)):
                        bg.pop(0)()
                        if bg and gi == 0 and slot not in _RESERVED:
                            bg.pop(0)()
                    rows = slice(64 * j, 64 * (j + 1))
                    ps_s = ps_pool.tile([128, 1024], F32, tag="s2", name="ps_s")
                    for half in range(2):
                        nc.tensor.matmul(
                            ps_s[:, 512 * half : 512 * (half + 1)],
                            kp_sb[rows, a, 128 * (2 * t + half) : 128 * (2 * t + half + 1)],
                            qp_sb[rows, a, 512 * qb : 512 * (qb + 1)],
                            start=True,
                            stop=True,
                        )
                    if blevel >= 1:
                        et = w_pool.tile([128, 1024], BF16, tag="et", bufs=4)
                        nc.scalar.activation(et[:], ps_s[:], Exp)
                    if blevel >= 2:
                        wt = w_pool.tile([128, 1024], BF16, tag="wt", bufs=16)
                        mul_idx[0] += 1
                        eng = (
                            nc.gpsimd
                            if POOL_MUL_NUM
                            and (mul_idx[0] * POOL_MUL_NUM) % POOL_MUL_DEN < POOL_MUL_NUM
                            else nc.vector
                        )
                        eng.tensor_mul(wt[:], et[:], em2[t][:])
                        wts[(t, j)] = wt
            prev, prev_wts = (attn_ps, a, qb), wts
        # tail: last group's remaining blocks + norms, then leftover bg
        if blevel >= 3 and prev is not None:
            p_ps, p_a, p_qb = prev
            attnv_block(p_ps[0], p_a, 0, 1, prev_wts)
            attnv_block(p_ps[1], p_a, 1, 1, prev_wts)
            if blevel >= 4:
                norm(p_ps[0], p_a, 0, p_qb)
                norm(p_ps[1], p_a, 1, p_qb)
        for u in bg:
            u()

    # ---- emission schedule ----
    has_a = "A" in phases
    has_b = "B" in phases
    has_c = "C" in phases

    def outproj_units(qb, copy_eng=None):
        return [
            (lambda oc=oc, qb=qb: outproj(oc, qb, copy_eng)) for oc in range(OC)
        ] if has_c else []

    # ---- DMA issue order: three queues in parallel.
    # SP: wk/xk/wv/xv/em00 ; ACT: wq/xq ; Pool SWDGE: emask + out.
    if has_a:
        xk0_xts = proj_dma("xk", 0, nc.sync)
        xq0_xts = proj_dma("xq", 0, nc.scalar)
    if has_b:
        em_prefetch(0)
    if has_a:
        xk1_xts = proj_dma("xk", 1, nc.sync)
        nc.sync.dma_start(wv_sb[:], d["wvT"])
        for kt in range(4):
            xv_dma(kt)
    nc.sync.dma_start(wo_sb[:], d["woT"])
    if has_b:
        em_prefetch(1)
    if has_a:
        xq1_xts = proj_dma("xq", 1, nc.scalar)

    # ---- compute emission
    if has_a:
        proj("xk", wk_sb, bk_sb, 1.0, kp_sb, 0, xts=xk0_xts)
        proj("xq", wq_sb, bq_sb, SCALE, qp_sb, 0, xts=xq0_xts)
    if has_b:
        bg0 = (
            proj_units(wk_sb, bk_sb, 1.0, kp_sb, 1, xk1_xts) + vproj_units()
            if has_a else []
        )

        def _g2():
            em_prefetch(2)
            return (
                proj_units(wq_sb, bq_sb, SCALE, qp_sb, 1, xq1_xts)
                if has_a else []
            )

        def _g4():
            em_prefetch(3)
            return []

        bg_append = {
            2: [_g2],
            3: [lambda: outproj_units(0)],
            4: [_g4],
            5: [lambda: outproj_units(1)],
            7: [lambda: outproj_units(2)],
        }
        attention(bg0, bg_append)
        for u in outproj_units(3):
            u()
    else:
        if has_a:
            proj("xk", wk_sb, bk_sb, 1.0, kp_sb, 1, xts=xk1_xts)
            for kt in range(KT):
                vproj_half(kt, 0)
                vproj_half(kt, 1)
            proj("xq", wq_sb, bq_sb, SCALE, qp_sb, 1, xts=xq1_xts)
        if has_c:
            for qb in range(QB):
                for u in outproj_units(qb):
                    u()


def build(repeat: int = 1, phases: str = "ABC", blevel: int = 4):
    nc = bacc.Bacc(
        "TRN2",
        target_bir_lowering=False,
        debug=False,
        enable_asserts=False,
        num_devices=NCORES,
    )
    d = {
        # x inputs packed: xq/xk as [qp2, ke, p, 1024] bf16
        "xq": nc.dram_tensor("xq", (QP2, KE, 128, 1024), BF16, kind="ExternalInput").ap(),
        "xk": nc.dram_tensor("xk", (QP2, KE, 128, 1024), BF16, kind="ExternalInput").ap(),
        # xv packed: [p, kt, ke, 256] bf16 (kt-chunked for early vproj)
        "xv": nc.dram_tensor("xv", (KT, 128, KE, 256), BF16, kind="ExternalInput").ap(),
        # emask packed: [kt, qb, p, 1024] where 1024 = (two, 512)
        "emask": nc.dram_tensor("emask", (KT, QB, 128, 1024), BF16, kind="ExternalInput").ap(),
        # weights packed: [p, ke, c] bf16 / wo [p, a, o] bf16
        "wqT": nc.dram_tensor("wqT", (128, KE, CH), BF16, kind="ExternalInput").ap(),
        "wkT": nc.dram_tensor("wkT", (128, KE, CH), BF16, kind="ExternalInput").ap(),
        "wvT": nc.dram_tensor("wvT", (128, KE, CH), BF16, kind="ExternalInput").ap(),
        "woT": nc.dram_tensor("woT", (128, A, E), BF16, kind="ExternalInput").ap(),
        "bq": nc.dram_tensor("bq", (128, A), F32, kind="ExternalInput").ap(),
        "bk": nc.dram_tensor("bk", (128, A), F32, kind="ExternalInput").ap(),
        # out packed: [oc, p, s]
        "out": nc.dram_tensor("out", (OC, 128, S), F32, kind="ExternalOutput").ap(),
    }
    with tile.TileContext(nc) as tc, ExitStack() as ctx:
        tc._emit_ctx = ctx
        if repeat == 1:
            _emit(tc, nc, d, phases, blevel)
        else:
            with tc.For_i(0, repeat, 1):
                _emit(tc, nc, d, phases, blevel)
    nc.compile()
    return nc


def _pack_x(x):  # (1024, S) f32 -> [qp2, ke, p, 1024] bf16
    bf = ml_dtypes.bfloat16
    a = x.reshape(KE, 128, QP2, 1024).transpose(2, 0, 1, 3)
    return np.ascontiguousarray(a).astype(bf)


def _pack_xv(x):  # (1024, S) f32 -> [kt, p, ke, 256] bf16
    bf = ml_dtypes.bfloat16
    a = x.reshape(KE, 128, KT, 256).transpose(2, 1, 0, 3)
    return np.ascontiguousarray(a).astype(bf)


def _pack_em(em):  # (S, S) f32 (already exp'd) -> [kt, qb, p, (two 512)] bf16
    bf = ml_dtypes.bfloat16
    a = em.reshape(KT, 2, 128, QB, 512).transpose(0, 3, 2, 1, 4).reshape(KT, QB, 128, 1024)
    return np.ascontiguousarray(a).astype(bf)


def _pack_w(wT):  # (E, CH) -> [p, ke, c] bf16
    bf = ml_dtypes.bfloat16
    a = wT.reshape(KE, 128, CH).transpose(1, 0, 2)
    return np.ascontiguousarray(a).astype(bf)


def _pack_wo(woT):  # (CH, E) -> [p, a, o] bf16
    bf = ml_dtypes.bfloat16
    a = woT.reshape(A, 128, E).transpose(1, 0, 2)
    return np.ascontiguousarray(a).astype(bf)


def _pack_b(b):  # (CH,) -> (128, A) f32
    return np.ascontiguousarray(b.reshape(A, 128).T).astype(np.float32)


def prep_inputs(q, k, v, qk_mask, Wq, bq, Wk, bk, Wv, bv, Wo, bo):
    q2 = np.asarray(q, np.float32).reshape(B, E, S)
    k2 = np.asarray(k, np.float32).reshape(B, E, S)
    v2 = np.asarray(v, np.float32).reshape(B, E, S)
    em = np.exp(np.asarray(qk_mask, np.float32).reshape(B, S, S))
    Wq = np.asarray(Wq, np.float32)
    Wk = np.asarray(Wk, np.float32)
    Wv = np.asarray(Wv, np.float32)
    Wo = np.asarray(Wo, np.float32)
    bqv = np.asarray(bq, np.float32)
    bkv = np.asarray(bk, np.float32)
    bvv = np.asarray(bv, np.float32)
    bov = np.asarray(bo, np.float32)
    host_bias = (Wo @ bvv + bov).astype(np.float32)

    xq = [_pack_x(q2[b]) for b in range(B)]
    xk = [_pack_x(k2[b]) for b in range(B)]
    xv = [_pack_xv(v2[b]) for b in range(B)]
    emp = [_pack_em(em[b]) for b in range(B)]

    in_maps = []
    for c in range(NCORES):
        b, g = divmod(c, GROUPS)
        ch = slice(CH * g, CH * (g + 1))
        in_maps.append(
            {
                "xq": xq[b],
                "xk": xk[b],
                "xv": xv[b],
                "emask": emp[b],
                "wqT": _pack_w(np.ascontiguousarray(Wq[ch].T)),
                "wkT": _pack_w(np.ascontiguousarray(Wk[ch].T)),
                "wvT": _pack_w(np.ascontiguousarray(Wv[ch].T)),
                "woT": _pack_wo(np.ascontiguousarray(Wo[:, ch].T)),
                "bq": _pack_b(bqv[ch]),
                "bk": _pack_b(bkv[ch]),
            }
        )
    return in_maps, host_bias


def unpack_out(packed):  # [oc, p, s] -> (E, S)
    return np.ascontiguousarray(packed.reshape(E, S))


_NC_CACHE = {}


def kernel(**inputs) -> np.ndarray:
    rep = int(os.environ.get("MHA_REPEAT", "1"))
    if rep not in _NC_CACHE:
        _NC_CACHE[rep] = build(rep)
    nc = _NC_CACHE[rep]
    in_maps, host_bias = prep_inputs(**inputs)
    res = bass_utils.run_bass_kernel_spmd(nc, in_maps, core_ids=list(range(NCORES)))
    out = np.zeros((B, E, 1, S), np.float32)
    for c in range(NCORES):
        b = c // GROUPS
        out[b, :, 0, :] += unpack_out(res.results[c]["out"])
    out += host_bias[None, :, None, None]
    return out
